# revision 3
# baseline (speedup 1.0000x reference)
"""Trainium2 Bass kernel v2: symmetric cyclic-window supervised-contrastive loss.

Exploits sim-matrix symmetry to halve the Scalar-engine exp work (the v1
bottleneck). Block grid: 130 blocks of 128 rows. Each block's strip computes
sim against a 66-block cyclic window (distances d in [0,65]); full row sums
are strip rowsums (fused ACT accum) plus column sums of other strips'
windows at d in [1,64], exchanged across the 8 cores with one AllReduce.
Pairs at d=65 are computed by both endpoints (rowsum only, no colsum).

SPMD uniformity: every core runs 17 strips over a locally-rotated feature
copy (local block l = global (R_c + l) mod 130). Cores 2-7 own 16 blocks;
their 17th strip uses a zeroed lhsT (separate input tensor) producing
constant exp(-C) colsum pollution that is corrected exactly via the
host-precomputed m_expdg. Local->global colsum realignment uses gpsimd
ap_gather with per-core index tensors.

q normalization, g label-sum vectors, diag/self terms, masks: host-prepped.
"""

import sys

import numpy as np

try:
    import concourse.bass as bass
except ImportError:
    sys.path.insert(0, "/opt/trn_rl_repo")
    import concourse.bass as bass

import concourse.bacc as bacc
import concourse.tile as tile
from concourse import mybir, bass_isa
from concourse.bass_utils import run_bass_kernel_spmd

AF = mybir.ActivationFunctionType
F32 = mybir.dt.float32
BF16 = mybir.dt.bfloat16
I16 = mybir.dt.int16

B, K, D = 256, 8192, 128
T = 0.07
C = 1.0 / T
N = B + 2 * K              # 16640
NB = N // 128              # 130 blocks
NCORES = 8
S = 17                     # strips per core (uniform program)
OWN = [17, 17, 16, 16, 16, 16, 16, 16]
RR = [0, 17, 34, 50, 66, 82, 98, 114]
WIN = 66                   # window width in blocks
NLOC = 82                  # local feature blocks stored
WCOLS = WIN * 128          # 8448
CHUNKS = [768, 1536, 1536, 1536, 1536, 1536]
NCH = len(CHUNKS)          # 6 ACT chunks per strip
ARW = 144                  # AllReduce buffer cols (130 padded to /16)
ACC_W = 96                 # local colsum accum cols (blocks 1..80 used)


def _patch_act_tables():
    """Prefer the table set holding BOTH exp and ln so the kernel pays one
    ACT_TABLE_LOAD instead of two (the second lands on the critical path
    right before the epilogue Ln)."""
    orig = bacc.get_activation_tables

    def filtered(arch):
        t = orig(arch)
        if "natural_log_exp_and_others" not in t:
            return t
        exp = mybir.ActivationFunctionType.Exp
        # same dict order (ids are positional); drop Exp from other sets so
        # the selector must pick the set that also holds Ln
        return {k: (v if k == "natural_log_exp_and_others" else v - {exp})
                for k, v in t.items()}

    bacc.get_activation_tables = filtered


def build_program():
    _patch_act_tables()
    nc = bacc.Bacc("TRN2", target_bir_lowering=False, debug=False,
                   enable_asserts=True, num_devices=NCORES)

    d_feat = nc.dram_tensor("featT", [D, NLOC * 128], BF16, kind="ExternalInput").ap()
    d_lhs16 = nc.dram_tensor("lhs16", [D, 128], BF16, kind="ExternalInput").ap()
    d_g = nc.dram_tensor("gvec", [D, 2], BF16, kind="ExternalInput").ap()
    d_ones = nc.dram_tensor("onesb", [D, 1], BF16, kind="ExternalInput").ap()
    d_mi = nc.dram_tensor("m_i1", [128, S], F32, kind="ExternalInput").ap()
    d_mr = nc.dram_tensor("m_rs", [128, S], F32, kind="ExternalInput").ap()
    d_mw = nc.dram_tensor("m_w", [128, S], F32, kind="ExternalInput").ap()
    d_mdg = nc.dram_tensor("m_dg", [128, S], F32, kind="ExternalInput").ap()
    d_medg = nc.dram_tensor("m_expdg", [128, S], F32, kind="ExternalInput").ap()
    d_ixp = nc.dram_tensor("idx_pre", [128, ARW // 16], I16, kind="ExternalInput").ap()
    d_id = nc.dram_tensor("ident", [128, 128], F32, kind="ExternalInput").ap()
    d_out = nc.dram_tensor("out", [128, 1], F32, kind="ExternalOutput").ap()

    from contextlib import ExitStack
    with tile.TileContext(nc) as tc, ExitStack() as ctx:
        feat = ctx.enter_context(tc.tile_pool(name="feat", bufs=1))
        consts = ctx.enter_context(tc.tile_pool(name="consts", bufs=1))
        accs = ctx.enter_context(tc.tile_pool(name="accs", bufs=1))
        exppool = ctx.enter_context(tc.tile_pool(name="exp", bufs=4))
        simpool = ctx.enter_context(tc.tile_pool(name="sim", bufs=2, space="PSUM"))
        pcpool = ctx.enter_context(tc.tile_pool(name="pc", bufs=2, space="PSUM"))
        dram = ctx.enter_context(tc.tile_pool(name="dram", bufs=2, space="DRAM"))

        # ---- DMAs: strip 0's first tiles in small chunks first so the PE
        # starts ~7us earlier; the bulk follows on other queues ----
        featT = feat.tile([D, NLOC * 128], BF16, tag="featT")
        NCOLS = NLOC * 128
        edges = list(range(0, 2048, 512)) + list(range(2048, 8448, 1600)) + \
            list(range(8448, NCOLS, 2048))
        edges.append(NCOLS)
        for a, b in zip(edges, edges[1:]):
            nc.sync.dma_start(out=featT[:, a:b], in_=d_feat[:, a:b])
        onesb = consts.tile([D, 1], BF16, tag="onesb")
        nc.sync.dma_start(out=onesb[:], in_=d_ones[:])
        lhs16 = consts.tile([D, 128], BF16, tag="lhs16")
        nc.sync.dma_start(out=lhs16[:], in_=d_lhs16[:])
        gvec = consts.tile([D, 2], BF16, tag="gvec")
        nc.sync.dma_start(out=gvec[:], in_=d_g[:])
        m_i1 = consts.tile([128, S], F32, tag="mi")
        nc.sync.dma_start(out=m_i1[:], in_=d_mi[:])
        m_rs = consts.tile([128, S], F32, tag="mr")
        nc.sync.dma_start(out=m_rs[:], in_=d_mr[:])
        m_w = consts.tile([128, S], F32, tag="mw")
        nc.sync.dma_start(out=m_w[:], in_=d_mw[:])
        m_dg = consts.tile([128, S], F32, tag="mdg")
        nc.sync.dma_start(out=m_dg[:], in_=d_mdg[:])
        m_edg = consts.tile([128, S], F32, tag="medg")
        nc.sync.dma_start(out=m_edg[:], in_=d_medg[:])
        ixp = consts.tile([128, ARW // 16], I16, tag="ixp")
        nc.sync.dma_start(out=ixp[:], in_=d_ixp[:])
        ident = consts.tile([128, 128], F32, tag="ident")
        nc.sync.dma_start(out=ident[:], in_=d_id[:])

        b_negC = consts.tile([128, 1], F32, tag="bnegC")
        nc.vector.memset(b_negC[:], -C)

        # tiny dummy AllReduce issued up front: warms the collectives
        # firmware path concurrently with the main compute so the real
        # AllReduce at the end starts/runs faster
        warm = consts.tile([128, 18], F32, tag="warm")
        nc.vector.memset(warm[:], 0.0)
        wcc_in = dram.tile([128, 18], F32)
        wcc_out = dram.tile([16, 18], F32)
        nc.sync.dma_start(out=wcc_in[:], in_=warm[:])
        for _ in range(2):
            nc.gpsimd.collective_compute(
                "ReduceScatter", mybir.AluOpType.add,
                replica_groups=[list(range(NCORES))],
                ins=[wcc_in.opt()], outs=[wcc_out.opt()],
            )

        acc = accs.tile([128, ACC_W], F32, tag="acc")
        nc.vector.memset(acc[:], 0.0)
        sums = accs.tile([128, S * NCH], F32, tag="sums")
        dg0 = accs.tile([128, S], F32, tag="dg0")
        dg1 = accs.tile([128, S], F32, tag="dg1")

        # ---- main loop (software-pipelined emission) ----
        # Colsum MMs for chunk (s,k) are emitted 2 chunks later in the PE
        # queue so the in-order PE never stalls waiting for ACT(s,k); the
        # g-dot MM has no exp dependency and is emitted at strip start.
        def lhsT_of(s):
            return lhs16[:] if s == 16 else featT[:, s * 128:(s + 1) * 128]

        work = []  # (s, pc, exps, w, base_d) awaiting colsum emission

        def emit_colsums(item):
            s_, pc_, exps_, w_, base_d = item
            for cc in range(w_ // 128):
                d = base_d + cc
                if 1 <= d <= 64:
                    nc.tensor.matmul(pc_[:, d - 1:d],
                                     exps_[:, cc * 128:(cc + 1) * 128],
                                     onesb[:], start=True, stop=True)
            if base_d + w_ // 128 == WIN:   # strip's last chunk: fold it
                nc.vector.tensor_add(acc[:, s_ + 1:s_ + 65], pc_[:, 0:64],
                                     acc[:, s_ + 1:s_ + 65])
                nc.vector.tensor_copy(dg0[:, s_:s_ + 1], pc_[:, 64:65])
                nc.vector.tensor_copy(dg1[:, s_:s_ + 1], pc_[:, 65:66])

        for s in range(S):
            lhsT = lhsT_of(s)
            pc = pcpool.tile([128, 66], F32, tag="pc")
            nc.tensor.matmul(pc[:, 64:66], lhsT, gvec[:], start=True, stop=True)
            off = s * 128
            pos = 0
            for k, w in enumerate(CHUNKS):
                ps = simpool.tile([128, 1536], F32, tag="ps")
                for j in range(0, w, 512):
                    mw = min(512, w - j)
                    nc.tensor.matmul(ps[:, j:j + mw], lhsT,
                                     featT[:, off + pos + j:off + pos + j + mw],
                                     start=True, stop=True)
                while len(work) >= 2:
                    emit_colsums(work.pop(0))
                exps = exppool.tile([128, 1536], BF16, tag="exps")
                nc.scalar.activation(exps[:, 0:w], ps[:, 0:w], AF.Exp,
                                     bias=b_negC[:], scale=1.0 / T,
                                     accum_out=sums[:, s * NCH + k:s * NCH + k + 1])
                work.append((s, pc, exps, w, pos // 128))
                pos += w
        for item in work:
            emit_colsums(item)

        # ---- RS-independent epilogue math (overlaps the collective) ----
        red = accs.tile([128, S], F32, tag="red")
        nc.vector.reduce_sum(red[:], sums[:].rearrange("p (t g) -> p t g", g=NCH),
                             axis=mybir.AxisListType.X)
        nc.vector.tensor_sub(red[:], red[:], m_edg[:])          # S1 minus colsums
        e1 = accs.tile([128, S], F32, tag="e1")
        nc.vector.tensor_sub(e1[:], dg1[:], dg0[:])
        nc.vector.tensor_mul(e1[:], e1[:], m_i1[:])
        nc.vector.tensor_add(e1[:], e1[:], dg0[:])              # dot(f_i, g_lab)
        nc.vector.tensor_sub(e1[:], e1[:], m_dg[:])
        nc.vector.tensor_mul(e1[:], e1[:], m_rs[:])             # S2/P term
        nc.vector.tensor_scalar_add(e1[:], e1[:], -C)

        # ---- colsum exchange: local -> global layout, AllReduce, extract ----
        garr = accs.tile([128, ARW], F32, tag="garr")
        nc.gpsimd.ap_gather(garr[:], acc[:], ixp[:], channels=128,
                            num_elems=ACC_W, d=1, num_idxs=ARW)
        # transpose garr to [144,128] so ReduceScatter shards along the
        # owner-grouped block axis; each core gets back only its own 18 rows
        tps = simpool.tile([128, 1536], F32, tag="ps")
        nc.tensor.transpose(tps[:, 0:128], garr[:, 0:128], ident[:])
        nc.tensor.transpose(tps[0:16, 128:256], garr[:, 128:144],
                            ident[:])
        gtr = accs.tile([128, 256], F32, tag="gtr")
        nc.vector.tensor_copy(gtr[:, 0:128], tps[:, 0:128])
        nc.vector.tensor_copy(gtr[0:16, 128:256], tps[0:16, 128:256])
        cc_in = dram.tile([ARW, 128], F32)
        cc_out = dram.tile([18, 128], F32)
        nc.sync.dma_start(out=cc_in[0:128, :], in_=gtr[:, 0:128])
        nc.sync.dma_start(out=cc_in[128:144, :], in_=gtr[0:16, 128:256])
        nc.gpsimd.collective_compute(
            "ReduceScatter", mybir.AluOpType.add,
            replica_groups=[list(range(NCORES))],
            ins=[cc_in.opt()], outs=[cc_out.opt()],
        )
        rsb = accs.tile([18, 128], F32, tag="rsb")
        nc.sync.dma_start(out=rsb[:], in_=cc_out[:])
        tps2 = simpool.tile([128, 1536], F32, tag="ps")
        nc.tensor.transpose(tps2[:, 0:18], rsb[:], ident[0:18, 0:18])
        colx = accs.tile([128, 32], F32, tag="colx")
        nc.vector.tensor_copy(colx[:, 0:18], tps2[:, 0:18])

        # ---- epilogue (post-RS critical path: add, Ln, sub, mul, reduce) ----
        nc.vector.tensor_add(red[:], red[:], colx[:, 0:S])      # S1
        lg = accs.tile([128, S], F32, tag="lg")
        nc.scalar.activation(lg[:], red[:], AF.Ln)
        nc.vector.tensor_sub(e1[:], e1[:], lg[:])               # contrib
        nc.vector.tensor_mul(e1[:], e1[:], m_w[:])
        outv = accs.tile([128, 1], F32, tag="outv")
        nc.vector.reduce_sum(outv[:], e1[:], axis=mybir.AxisListType.X)
        nc.sync.dma_start(out=d_out[:], in_=outv[:])

    nc.compile()
    return nc


def _wrap_idx(flat):
    """Pack a flat index list into the ap_gather [128, n/16] wrapped layout."""
    n = len(flat)
    assert n % 16 == 0
    a = np.asarray(flat, np.int16).reshape(n // 16, 16).T   # [16, n/16]
    return np.ascontiguousarray(np.tile(a, (8, 1)))


def prep_in_maps(q, ba_queue, nonba_queue, targets):
    import ml_dtypes
    BF = ml_dtypes.bfloat16
    q = np.asarray(q, dtype=np.float32)
    ba = np.asarray(ba_queue, dtype=np.float32)
    nb = np.asarray(nonba_queue, dtype=np.float32)
    tg = np.asarray(targets).astype(np.int64)

    qn = q / np.clip(np.linalg.norm(q, axis=1, keepdims=True), 1e-12, None)
    fullT_bf = np.concatenate([qn.T, ba.T, nb.T], axis=1).astype(BF)  # [128,N]
    f32 = fullT_bf.astype(np.float32)
    labels = np.concatenate([tg, np.ones(K, np.int64), np.zeros(K, np.int64)])
    dvec = (f32 * f32).sum(axis=0)
    c1 = int(labels.sum())
    c0 = N - c1
    P = np.where(labels == 1, c1 - 1, c0 - 1).astype(np.float64)
    g = np.stack([f32[:, labels == 0].astype(np.float64).sum(axis=1),
                  f32[:, labels == 1].astype(np.float64).sum(axis=1)], axis=1)
    g_bf = np.ascontiguousarray(g.astype(BF))

    # dummy-strip colsum pollution correction per global block
    corr = np.zeros(NB)
    for c in range(NCORES):
        if OWN[c] == 17:
            continue
        for dd in range(1, 65):
            corr[(RR[c] + 16 + dd) % NB] += 128.0 * np.exp(-C)

    in_maps = []
    for c in range(NCORES):
        featT = np.roll(fullT_bf, -RR[c] * 128, axis=1)[:, :NLOC * 128]
        lhs16 = (featT[:, 16 * 128:17 * 128] if OWN[c] == 17
                 else np.zeros((D, 128), BF))
        mi = np.zeros((128, S), np.float32)
        mr = np.ones((128, S), np.float32)
        mw = np.zeros((128, S), np.float32)
        mdg = np.zeros((128, S), np.float32)
        medg = np.zeros((128, S), np.float32)
        for s in range(min(S, OWN[c])):
            gblk = (RR[c] + s) % NB
            rows = slice(gblk * 128, gblk * 128 + 128)
            mi[:, s] = labels[rows]
            mr[:, s] = 1.0 / (T * P[rows])
            mw[:, s] = -1.0 / N
            mdg[:, s] = dvec[rows]
            medg[:, s] = np.exp(dvec[rows] / T - C) + corr[gblk]
        # owner-grouped layout: position p*18+j holds global block RR[p]+j
        idx_pre = []
        for t in range(ARW):
            p, j = t // 18, t % 18
            if j < OWN[p]:
                loc = ((RR[p] + j) - RR[c]) % NB
                idx_pre.append(loc if 1 <= loc <= 80 else 0)
            else:
                idx_pre.append(0)
        in_maps.append({
            "featT": np.ascontiguousarray(featT),
            "lhs16": np.ascontiguousarray(lhs16),
            "gvec": g_bf,
            "onesb": np.ones((D, 1), BF),
            "m_i1": mi, "m_rs": mr, "m_w": mw, "m_dg": mdg, "m_expdg": medg,
            "idx_pre": _wrap_idx(idx_pre),
            "ident": np.eye(128, dtype=np.float32),
        })
    return in_maps


_PROGRAM = None


def get_program():
    global _PROGRAM
    if _PROGRAM is None:
        _PROGRAM = build_program()
    return _PROGRAM


def run_on_hw(in_maps, trace=False):
    return run_bass_kernel_spmd(get_program(), in_maps, list(range(NCORES)),
                                trace=trace)


def kernel(q, ba_queue, nonba_queue, targets):
    in_maps = prep_in_maps(q, ba_queue, nonba_queue, targets)
    res = run_on_hw(in_maps)
    total = sum(float(r["out"].astype(np.float64).sum()) for r in res.results)
    return np.array(total, dtype=np.float32)
